# revision 1
# baseline (speedup 1.0000x reference)
"""Block-sparse (DeepSpeed fixed-layout) causal self-attention on 8 trn2 NeuronCores.

Problem: B=2, H=16, L=2048, D=64, fp32; BLOCK=16, STRIDE=64, NUMVERTS=1, VERTSIZE=1.
Layout per head (identical for all heads since numverts=1):
  - intra-window block-causal attention within each 64-token window (4 blocks of 16)
  - "summary" attention: every query attends the last 16 tokens (block col 3) of
    every *earlier* 64-token window.

Strategy (per core; 32 (b,h) pairs sharded 4 per core, no collectives):
  S^T dataflow:  St[k,q] = lhsT.T @ rhs with
     lhsT = [K^T ; mask-selector rows]  (stationary, fp16)
     rhs  = [Q^T/8 ; mask-value rows]   (moving, fp16)
  so the additive -30000 masks are fused into the QK matmul as extra contraction
  rows (rank-4 local causal mask + rank-8-per-chunk triangular summary masks).
  exp() on ScalarE (PSUM fp32 -> SBUF fp16, no max-subtraction needed: |scores|<~7).
  AV: out[q,d] = Et.T @ [V | 1]  -- Et (fp16) is the stationary operand, V carries a
  ones column so column 64 of the PSUM output is the softmax denominator l[q].
  The device ships unnormalized [O_unnorm | l] (DVE 2x-mode copy PSUM->SBUF, one
  DMA per (b,h)); the final O = O_unnorm / l division is host-side numpy, like
  all other layout work (Q^T/K^T transposes, summary gathers, mask constants).
  No transposes and no reductions anywhere on device.
"""

import os
import numpy as np

# ---------------- problem constants (hardcoded per contract) ----------------
B, H, L, D = 2, 16, 2048, 64
BLOCK = 16
WIN = 64              # stride window (tokens)
NWIN = L // WIN       # 32 windows
NSUM = NWIN * BLOCK   # 512 summary keys (last 16 tokens of each window)
NG = 4                # query groups per sequence
GQ = L // NG          # 512 queries per group
NCORES = 8
NBH = (B * H) // NCORES  # 4 (b,h) per core
KP = 128              # contraction partitions: 64 d + 4 local mask + 32 tri mask + 28 zero
MASKVAL = -30000.0

_SUMIDX = np.array([64 * m + 48 + j for m in range(NWIN) for j in range(BLOCK)])


def _host_masks():
    """Constant mask rows appended to the contraction dim. fp16.

    Local attention is computed per *pair* of windows (128 keys x 128 queries).
    mq [64, L]    : mask *values* rows (appended to Q^T, the moving operand)
                    rows 0-7   = V8pair local-causal values (periodic 128)
                    rows 8-39  = V8_s triangular summary values (s = 0..3)
                    rows 40-63 = 0
    mk [64, L]    : mask *selector* rows appended to K^T (local stationary)
                    rows 0-7   = U8 one-hot of key 16-block within window pair
    ms [64, NSUM] : selector rows appended to the gathered summary K^T
                    rows 8+8s+b = one-hot of summary chunk s, block b
    """
    qc = np.arange(L)
    j = qc % 128          # query col within pair
    ap = j // WIN         # query window within pair (0/1)
    rp = (j % WIN) // BLOCK
    mq = np.zeros((64, L), np.float32)
    for i in range(8):
        a, b = i // 4, i % 4
        active = ((a == ap) & (b <= rp)) | ((a == 0) & (ap == 1) & (b == 3))
        mq[i] = np.where(active, 0.0, MASKVAL)
    for s in range(4):
        for b in range(8):
            # summary block m=8s+b masked for q in group s with pair idx <= b//2
            mq[8 + 8 * s + b] = np.where(
                (qc // GQ == s) & ((qc % GQ) // 128 <= b // 2), MASKVAL, 0.0
            )
    mk = np.zeros((64, L), np.float32)
    kc = np.arange(L)
    for i in range(8):
        mk[i] = ((kc % 128) // BLOCK == i).astype(np.float32)
    sc = np.arange(NSUM)
    ms = np.zeros((64, NSUM), np.float32)
    for s in range(4):
        for b in range(8):
            ms[8 + 8 * s + b] = ((sc // 128 == s) & ((sc % 128) // BLOCK == b)).astype(
                np.float32
            )
    return mq.astype(np.float16), mk.astype(np.float16), ms.astype(np.float16)


# ---------------- device program ----------------
_NC_CACHE = {}


def _build_nc():
    if "nc" in _NC_CACHE:
        return _NC_CACHE["nc"]
    from contextlib import ExitStack

    import concourse.bacc as bacc
    import concourse.bass as bass
    import concourse.tile as tile
    from concourse import mybir

    F16 = mybir.dt.float16
    F32 = mybir.dt.float32
    EXP = mybir.ActivationFunctionType.Exp

    nc = bacc.Bacc("TRN2", target_bir_lowering=False)

    # qkt = [Q^T/8 | K^T | gathered-summary K^T] concatenated along cols
    qkt_d = nc.dram_tensor("qkt", [NBH, 64, 2 * L + NSUM], F16, kind="ExternalInput")
    # vpx = [V|1] reshaped (16 local 128-key tiles) ++ gathered summary [V|1]
    # (4 tiles) -> one tensor, one DMA per (b,h)
    vpx_d = nc.dram_tensor("vpx", [NBH, 128, 20, 65], F16, kind="ExternalInput")
    mall_d = nc.dram_tensor("mall", [64, 2 * L + NSUM], F16, kind="ExternalInput")
    # unnormalized output; col 64 = softmax denominator l (host divides)
    o_d = nc.dram_tensor("o", [NBH, L, 65], F32, kind="ExternalOutput")

    with tile.TileContext(nc) as tc, ExitStack() as ctx:
        const = ctx.enter_context(tc.tile_pool(name="const", bufs=1))
        inbuf = ctx.enter_context(tc.tile_pool(name="inbuf", bufs=2))
        etp = ctx.enter_context(tc.tile_pool(name="etp", bufs=3))
        etsum = ctx.enter_context(tc.tile_pool(name="etsum", bufs=6))
        psum = ctx.enter_context(tc.tile_pool(name="psum", bufs=2, space="PSUM"))
        outp = ctx.enter_context(tc.tile_pool(name="outp", bufs=2))

        # double-buffered wide base [Q^T | K^T | KTS] with persistent mask rows
        qktb = [const.tile([KP, 2 * L + NSUM], F16, name=f"qktb{j}") for j in range(2)]
        for j in range(2):
            nc.sync.dma_start(out=qktb[j][64:128, :], in_=mall_d.ap())

        for i in range(NBH):
            qkt = qktb[i % 2]
            nc.sync.dma_start(out=qkt[0:64, :], in_=qkt_d.ap()[i])
            qt = qkt[:, 0:L]
            kt = qkt[:, L : 2 * L]
            kts = qkt[:, 2 * L : 2 * L + NSUM]
            vpx = inbuf.tile([128, 20, 65], F16, tag="vpx")
            nc.sync.dma_start(out=vpx, in_=vpx_d.ap()[i])
            osb = outp.tile([128, 16, 65], F32, tag="osb", name=f"osb_{i}")

            for g in range(NG):
                # ---- summary QK + exp (chunks s = 0..g of 128 summary keys) ----
                ets = []
                for s in range(g + 1):
                    st = psum.tile(
                        [128, GQ], F32, tag="st_sum", name=f"st_{i}_{g}_{s}", bufs=3
                    )
                    nc.tensor.matmul(
                        st,
                        kts[:, 128 * s : 128 * (s + 1)],
                        qt[:, GQ * g : GQ * (g + 1)],
                        start=True,
                        stop=True,
                    )
                    e = etsum.tile([128, GQ], F16, tag="et_sum", name=f"et_{i}_{g}_{s}")
                    nc.scalar.activation(out=e, in_=st, func=EXP)
                    ets.append(e)

                # ---- local QK (4 window-pairs) + exp ----
                stl = psum.tile([128, 512], F32, tag="st_loc", name=f"stl_{i}_{g}")
                for u in range(4):
                    p = 4 * g + u
                    nc.tensor.matmul(
                        stl[:, 128 * u : 128 * (u + 1)],
                        kt[:, 128 * p : 128 * (p + 1)],
                        qt[:, 128 * p : 128 * (p + 1)],
                        start=True,
                        stop=True,
                        skip_group_check=True,
                    )
                etl = etp.tile([128, 512], F16, tag="et_loc", name=f"etl_{i}_{g}")
                nc.scalar.activation(out=etl, in_=stl, func=EXP)

                # ---- AV per 128-query chunk ----
                op = psum.tile(
                    [128, 512], F32, tag="opsum", name=f"op_{i}_{g}", bufs=3
                )
                op_r = op.rearrange("p (t c) -> p t c", c=128)
                for tq in range(4):
                    t = 4 * g + tq
                    nc.tensor.matmul(
                        op_r[:, tq, 0:65],
                        etl[:, 128 * tq : 128 * tq + 128],
                        vpx[:, t, :],
                        start=True,
                        stop=False,
                        skip_group_check=True,
                    )
                    for s in range(g + 1):
                        nc.tensor.matmul(
                            op_r[:, tq, 0:65],
                            ets[s][:, 128 * tq : 128 * tq + 128],
                            vpx[:, 16 + s, :],
                            start=False,
                            stop=(s == g),
                            skip_group_check=True,
                        )

                # ---- move unnormalized O + l to SBUF (host divides) ----
                nc.vector.tensor_copy(
                    out=osb[:, 4 * g : 4 * g + 4, :], in_=op_r[:, :, 0:65]
                )
            dst = o_d.ap()[i].rearrange("(t p) c -> p t c", p=128)
            nc.sync.dma_start(out=dst, in_=osb)

    nc.compile()
    _NC_CACHE["nc"] = nc
    return nc


def _prep_core_inputs(qf, kf, vf, bhs, mq, mk, ms):
    """Build one core's input dict from flat [32, L, D] fp32 arrays."""
    qkt = np.empty((NBH, 64, 2 * L + NSUM), np.float16)
    vpx = np.empty((NBH, 128, 20, 65), np.float16)
    for j, bh in enumerate(bhs):
        qkt[j, :, 0:L] = (qf[bh].T * 0.125).astype(np.float16)
        qkt[j, :, L : 2 * L] = kf[bh].T.astype(np.float16)
        qkt[j, :, 2 * L :] = kf[bh][_SUMIDX].T.astype(np.float16)
        vp1 = np.concatenate([vf[bh], np.ones((L, 1), np.float32)], axis=1).astype(
            np.float16
        )
        vpx[j, :, :16, :] = vp1.reshape(16, 128, 65).transpose(1, 0, 2)
        vs1 = np.concatenate(
            [vf[bh][_SUMIDX], np.ones((NSUM, 1), np.float32)], axis=1
        ).astype(np.float16)
        vpx[j, :, 16:, :] = vs1.reshape(4, 128, 65).transpose(1, 0, 2)
    mall = np.concatenate([mq, mk, ms], axis=1)
    return {"qkt": qkt, "vpx": vpx, "mall": mall}


def _finish(o_raw):
    """[n, L, 65] unnormalized device output -> [n, L, 64] normalized."""
    o_raw = np.asarray(o_raw, np.float32)
    return o_raw[..., :64] / o_raw[..., 64:65]


def _in_maps(query, key, value):
    qf = np.asarray(query, np.float32).reshape(B * H, L, D)
    kf = np.asarray(key, np.float32).reshape(B * H, L, D)
    vf = np.asarray(value, np.float32).reshape(B * H, L, D)
    mq, mk, ms = _host_masks()
    return [
        _prep_core_inputs(qf, kf, vf, range(NBH * c, NBH * (c + 1)), mq, mk, ms)
        for c in range(NCORES)
    ]


def kernel(query, key, value):
    from concourse.bass_utils import run_bass_kernel_spmd

    nc = _build_nc()
    res = run_bass_kernel_spmd(nc, _in_maps(query, key, value), list(range(NCORES)))
    out = np.concatenate([_finish(res.results[c]["o"]) for c in range(NCORES)])
    return out.reshape(B, H, L, D).astype(np.float32)



# revision 30
# speedup vs baseline: 3.6895x; 3.6895x over previous
"""Block-sparse (DeepSpeed fixed-layout) causal self-attention on 8 trn2 NeuronCores.

Problem: B=2, H=16, L=2048, D=64, fp32; BLOCK=16, STRIDE=64, NUMVERTS=1, VERTSIZE=1.
Layout per head (identical for all heads since numverts=1):
  - intra-window block-causal attention within each 64-token window (4 blocks of 16)
  - "summary" attention: every query attends the last 16 tokens (block col 3) of
    every *earlier* 64-token window.

Strategy (per core; 32 (b,h) pairs sharded 4 per core, no collectives).
The Activation engine (exp, ~0.83 ns/column + ~143 ns/instr PSUM-access
penalty) is the binding resource, so the design minimizes exp'd columns and
exp instruction count; the PE side minimizes matmul instruction count
(small-N matmuls are issue-floor-bound):

  Summary QK:  St[sk,q] = lhsT.T @ rhs with
     lhsT = [summary K^T ; one-hot selector rows]   (stationary, fp16)
     rhs  = [Q^T/8 ; per-(window,block) mask-value rows]  (moving, fp16)
  -- the window-granularity causal masks are fused into the matmul as rank-8
  contraction rows per 128-key chunk.  The "active" mask value is -6*ln2 (not
  0) so exp() output is scaled 2^-6, keeping the unnormalized result in fp16
  range.  Chunks are computed into 2-bank PSUM pieces and exp'd with one
  fused-AP activation per piece (half the per-instr penalty of per-chunk exp).

  Packed local QK: the two 64-token windows of each window pair are computed
  by concurrent 64x64 PE tiles at tile_position (0,0)/(64,64) (even windows
  from partitions 0-63, odd windows from a second Q/K copy in partitions
  64-127), packing both windows' scores into one [128, 64] PSUM column range.
  This HALVES the local exp columns ([128,256] per group instead of [128,512]).
  The in-window causal mask (x 2^-6) is a DVE multiply producing two
  half-masked copies (etmA zeroes partitions 64-127, etmB zeroes 0-63) so the
  AV can contract the full 128 partitions with plain untiled matmuls --
  concurrent row-tiled matmuls would collide on the PSUM write port (HW fault).

  AV is transposed:  Ot[d,q] = [V|1].T @ Et  -- V (65 cols: 64 d + ones) is
  the *stationary* operand, the exp'd scores stream as the moving operand.
  This cuts AV to 18 matmuls per (b,h) and accumulates local + all summary
  chunks straight into one [65, 512] PSUM bank per query group.  Row 64 of Ot
  is the softmax denominator l.

  The device ships unnormalized Ot = [O^T ; l^T] fp16 (DVE copy PSUM->SBUF,
  per-group Pool-queue DMA); the host transposes and divides:
  O = (Ot[:64]/Ot[64]).T.  Startup DMAs land the first summary matmuls'
  operands ([summary K | group-0 Q] prefix) before the bulk; the last (b,h)
  runs its groups in reverse so the kernel ends on the shortest AV chain.
"""

import os
import numpy as np

# ---------------- problem constants (hardcoded per contract) ----------------
B, H, L, D = 2, 16, 2048, 64
BLOCK = 16
WIN = 64              # stride window (tokens)
NWIN = L // WIN       # 32 windows
NSUM = NWIN * BLOCK   # 512 summary keys (last 16 tokens of each window)
NG = 4                # query groups per sequence
GQ = L // NG          # 512 queries per group
NCORES = 8
NBH = (B * H) // NCORES  # 4 (b,h) per core
KP = 128              # contraction partitions: 64 d + 4 local mask + 32 tri mask + 28 zero
MASKVAL = -30000.0
EBIAS = -6.0 * float(np.log(2.0))  # exp() scale 2^-6: keeps unnorm output in fp16

_SUMIDX = np.array([64 * m + 48 + j for m in range(NWIN) for j in range(BLOCK)])


def _host_masks():
    """Constant mask rows appended to the contraction dim. fp16.

    Local attention is computed per *pair* of windows (128 keys x 128 queries).
    mq [64, L]    : mask *values* rows (appended to Q^T, the moving operand)
                    rows 0-7   = V8pair local-causal values (periodic 128)
                    rows 8-39  = V8_s triangular summary values (s = 0..3)
                    rows 40-63 = 0
    mk [64, L]    : mask *selector* rows appended to K^T (local stationary)
                    rows 0-7   = U8 one-hot of key 16-block within window pair
    ms [64, NSUM] : selector rows appended to the gathered summary K^T
                    rows 8+8s+b = one-hot of summary chunk s, block b
    Active (unmasked) entries carry EBIAS instead of 0 so every exp() result is
    scaled by 2^-6 (exactly one selector row fires per key column).
    """
    qc = np.arange(L)
    mq = np.zeros((64, L), np.float32)
    for s in range(4):
        for b in range(8):
            # summary block m=8s+b masked for queries in window <= m
            # (window granularity: same-window handled by the local path)
            mq[8 + 8 * s + b] = np.where(8 * s + b >= qc // WIN, MASKVAL, EBIAS)
    mk = np.zeros((64, L), np.float32)
    sc = np.arange(NSUM)
    ms = np.zeros((64, NSUM), np.float32)
    for s in range(4):
        for b in range(8):
            ms[8 + 8 * s + b] = ((sc // 128 == s) & ((sc % 128) // BLOCK == b)).astype(
                np.float32
            )
    return mq.astype(np.float16), mk.astype(np.float16), ms.astype(np.float16)


def _host_m01():
    """[128, 512] fp16 packed-local causal masks: partition p = key offset in
    window (two windows stacked), col j = (pair-in-group, query offset).
    Value 2^-6 where key block <= query block (the exp() 2^-6 scale rides the
    mask since packed-local scores carry no bias rows).  Cols 0:256 mask the
    A-window half (partitions 64-127 zeroed), cols 256:512 the B half -- the
    AV then contracts the full 128 partitions with plain untiled matmuls
    (concurrent row-tiled matmuls collide on the PSUM write port)."""
    p = np.arange(128)[:, None]
    j = np.arange(256)[None, :]
    valid = (p % 64) // BLOCK <= (j % 64) // BLOCK
    m = np.where(valid, 2.0 ** -6, 0.0)
    ma = np.where(p < 64, m, 0.0)
    mb = np.where(p >= 64, m, 0.0)
    return np.concatenate([ma, mb], axis=1).astype(np.float16)


# ---------------- device program ----------------
_NC_CACHE = {}


def _build_nc(reps=1):
    """reps>1 repeats the whole computation in-NEFF (timing only)."""
    if reps in _NC_CACHE:
        return _NC_CACHE[reps]
    from contextlib import ExitStack

    import concourse.bacc as bacc
    import concourse.bass as bass
    import concourse.tile as tile
    from concourse import mybir

    F16 = mybir.dt.float16
    F32 = mybir.dt.float32
    EXP = mybir.ActivationFunctionType.Exp

    nc = bacc.Bacc("TRN2", target_bir_lowering=False)

    # qkt = [gathered-summary K^T | Q^T/8 | K^T] concatenated along cols
    # (summary K first so the startup-critical DMA is one contiguous prefix)
    qkt_d = nc.dram_tensor("qkt", [NBH, 64, 2 * L + NSUM], F16, kind="ExternalInput")
    # vpx = [V|1] reshaped (16 local 128-key tiles) ++ gathered summary [V|1]
    # (4 tiles) -> one tensor, one DMA per (b,h)
    vpx_d = nc.dram_tensor("vpx", [NBH, 128, 20, 65], F16, kind="ExternalInput")
    mall_d = nc.dram_tensor("mall", [64, 2 * L + NSUM], F16, kind="ExternalInput")
    # odd-window Q^T/8 ++ K^T, landed in partitions 64-127 of the K region
    # (the packed-local 64x64 PE tiles need both windows of a pair in
    # disjoint partition halves)
    qko_d = nc.dram_tensor("qko", [NBH, 64, 2 * L // 2], F16, kind="ExternalInput")
    m01_d = nc.dram_tensor("m01", [128, 512], F16, kind="ExternalInput")
    # unnormalized transposed output; row 64 = softmax denominator l (host
    # transposes + divides)
    o_d = nc.dram_tensor("o", [NBH, 65, L], F16, kind="ExternalOutput")

    with tile.TileContext(nc) as tc, ExitStack() as ctx:
        const = ctx.enter_context(tc.tile_pool(name="const", bufs=1))
        inbuf = ctx.enter_context(tc.tile_pool(name="inbuf", bufs=2))
        etp = ctx.enter_context(tc.tile_pool(name="etp", bufs=3))
        etsum = ctx.enter_context(tc.tile_pool(name="etsum", bufs=2))
        psum = ctx.enter_context(tc.tile_pool(name="psum", bufs=2, space="PSUM"))
        outp = ctx.enter_context(tc.tile_pool(name="outp", bufs=2))

        # double-buffered wide base [Q^T | K^T | KTS] with persistent mask rows
        qktb = [const.tile([KP, 2 * L + NSUM], F16, name=f"qktb{j}") for j in range(2)]
        m01t = const.tile([128, 512], F16, name="m01t")
        nc.gpsimd.dma_start(out=m01t, in_=m01_d.ap())

        def bh_setup(rep, i):
            """Emit input DMAs for (rep, i); return the per-bh view dict.
            The mask rows (mall) only cover [0 : NSUM+L) -- the old mk region
            is dead and overwritten by qko, and overlapping DMAs would chain
            a WAW wait into the SP DMA FIFO."""
            qkt = qktb[i % 2]
            if rep == 0 and i == 0:
                # critical-path startup: land the prefix the first summary
                # matmuls need ([KTS | Q group 0] + its mask rows) first
                src = qkt_d.ap()[i]
                msrc = mall_d.ap()
                cut = NSUM + GQ
                nc.scalar.dma_start(out=qkt[64:128, 0:cut], in_=msrc[:, 0:cut])
                nc.sync.dma_start(out=qkt[0:64, 0:cut], in_=src[:, 0:cut])
                nc.sync.dma_start(
                    out=qkt[64:128, cut : NSUM + L], in_=msrc[:, cut : NSUM + L]
                )
                # group-0 local QK needs the odd-window data early
                nc.sync.dma_start(
                    out=qkt[64:128, NSUM + L : NSUM + 2 * L], in_=qko_d.ap()[i]
                )
                nc.sync.dma_start(out=qkt[0:64, cut:], in_=src[:, cut:])
            else:
                if rep == 0 and i == 1:
                    # first use of the second buffer: load its mask rows
                    nc.sync.dma_start(
                        out=qkt[64:128, 0 : NSUM + L], in_=mall_d.ap()[:, 0 : NSUM + L]
                    )
                nc.sync.dma_start(out=qkt[0:64, :], in_=qkt_d.ap()[i])
            if not (rep == 0 and i == 0):
                nc.sync.dma_start(
                    out=qkt[64:128, NSUM + L : NSUM + 2 * L], in_=qko_d.ap()[i]
                )
            vpx = inbuf.tile([128, 20, 65], F16, tag="vpx")
            nc.sync.dma_start(out=vpx, in_=vpx_d.ap()[i])
            osb = outp.tile([128, NG, GQ], F16, tag="osb", name=f"osb_{rep}_{i}")
            return dict(
                kts=qkt[:, 0:NSUM],
                qt=qkt[:, NSUM : NSUM + L],
                kt=qkt[:, NSUM + L : NSUM + 2 * L],
                qo=qkt[:, NSUM + L : NSUM + L + L // 2],      # partitions 64-127
                ko=qkt[:, NSUM + L + L // 2 : NSUM + 2 * L],  # partitions 64-127
                vpx=vpx,
                osb=osb,
            )

        # software-pipelined (b,h) x group unit stream: the next bh's first
        # (lightest) group is emitted before the current bh's last (heaviest)
        # group so ACT never starves at bh boundaries; the final bh runs its
        # groups in reverse so the kernel ends on the shortest AV chain.
        def unit_stream(rep):
            stream = []
            for i in range(NBH):
                gs = [0, 1, 2, 3] if i < NBH - 1 else [3, 2, 1, 0]
                stream.extend((i, g) for g in gs)
            return stream

        for rep in range(reps):
            views = {}
            for i, g in unit_stream(rep):
                if i not in views:
                    views[i] = bh_setup(rep, i)
                v = views[i]
                kts, qt, kt, qo, ko = v["kts"], v["qt"], v["kt"], v["qo"], v["ko"]
                vpx, osb = v["vpx"], v["osb"]
                # ---- summary QK (chunks s = 0..g) + exp fused per 2-chunk
                # piece: halves ACT's per-instr PSUM access penalty while
                # keeping the pipeline fine-grained ----
                ets = []
                for a in range(0, g + 1, 2):
                    b = min(a + 2, g + 1)
                    st = psum.tile(
                        [128, 2, GQ], F32, tag="st_sum",
                        name=f"st_{rep}_{i}_{g}_{a}", bufs=2,
                    )
                    for s in range(a, b):
                        nc.tensor.matmul(
                            st[:, s - a, :],
                            kts[:, 128 * s : 128 * (s + 1)],
                            qt[:, GQ * g : GQ * (g + 1)],
                            start=True,
                            stop=True,
                            skip_group_check=True,
                        )
                    e = etsum.tile(
                        [128, 2, GQ], F16, tag="et_sum",
                        name=f"et_{rep}_{i}_{g}_{a}", bufs=4,
                    )
                    nc.scalar.activation(
                        out=e[:, 0 : b - a, :], in_=st[:, 0 : b - a, :], func=EXP
                    )
                    ets.extend(e[:, s - a, :] for s in range(a, b))

                # ---- packed local QK: two 64-token windows per pair in
                # disjoint partition halves via 64x64 PE tiles (halves the
                # local exp columns) ----
                stl = psum.tile([128, 256], F32, tag="st_loc", name=f"stl_{rep}_{i}_{g}")
                for u in range(4):
                    p = 4 * g + u
                    nc.tensor.matmul(
                        stl[0:64, 64 * u : 64 * (u + 1)],
                        kt[0:64, 128 * p : 128 * p + 64],
                        qt[0:64, 128 * p : 128 * p + 64],
                        start=True,
                        stop=True,
                        skip_group_check=True,
                    )
                    nc.tensor.matmul(
                        stl[64:128, 64 * u : 64 * (u + 1)],
                        ko[64:128, 64 * p : 64 * (p + 1)],
                        qo[64:128, 64 * p : 64 * (p + 1)],
                        start=True,
                        stop=True,
                        skip_group_check=True,
                    )
                etl = etp.tile([128, 256], F16, tag="et_loc", name=f"etl_{rep}_{i}_{g}")
                nc.scalar.activation(out=etl, in_=stl, func=EXP)
                # causal-in-window masks (x 2^-6 exp scale), DVE 4x mode;
                # A/B halves also zero the other window's partitions
                etma = etp.tile([128, 256], F16, tag="et_mska", name=f"etma_{rep}_{i}_{g}")
                nc.vector.tensor_mul(etma, etl, m01t[:, 0:256])
                etmb = etp.tile([128, 256], F16, tag="et_mskb", name=f"etmb_{rep}_{i}_{g}")
                nc.vector.tensor_mul(etmb, etl, m01t[:, 256:512])

                # ---- transposed AV: Ot[65, 512] = [V|1].T @ Et, V stationary ----
                # start=True only on the first matmul: it clears the whole
                # bank's has_written bits; u>0 then overwrite their (cleared)
                # ranges with start=False, and the summary matmuls accumulate.
                ot = psum.tile(
                    [128, GQ], F32, tag="otps", name=f"ot_{rep}_{i}_{g}", bufs=2
                )
                for u in range(4):
                    nc.tensor.matmul(
                        ot[0:65, 128 * u : 128 * u + 64],
                        vpx[:, 4 * g + u, :],
                        etma[:, 64 * u : 64 * (u + 1)],
                        start=(u == 0),
                        stop=False,
                        skip_group_check=True,
                    )
                    nc.tensor.matmul(
                        ot[0:65, 128 * u + 64 : 128 * (u + 1)],
                        vpx[:, 4 * g + u, :],
                        etmb[:, 64 * u : 64 * (u + 1)],
                        start=False,
                        stop=False,
                        skip_group_check=True,
                    )
                for s in range(g + 1):
                    nc.tensor.matmul(
                        ot[0:65, :],
                        vpx[:, 16 + s, :],
                        ets[s],
                        start=False,
                        stop=(s == g),
                        skip_group_check=True,
                    )

                # ---- move unnormalized Ot to SBUF fp16 (host divides) ----
                nc.vector.tensor_copy(out=osb[0:65, g, :], in_=ot[0:65, :])
                nc.gpsimd.dma_start(
                    out=o_d.ap()[i][:, GQ * g : GQ * (g + 1)], in_=osb[0:65, g, :]
                )

    nc.compile()
    _NC_CACHE[reps] = nc
    return nc


def _prep_core_inputs(qf, kf, vf, bhs, mq, mk, ms):
    """Build one core's input dict from flat [32, L, D] fp32 arrays."""
    qkt = np.empty((NBH, 64, 2 * L + NSUM), np.float16)
    qko = np.empty((NBH, 64, L), np.float16)
    vpx = np.empty((NBH, 128, 20, 65), np.float16)
    for j, bh in enumerate(bhs):
        qt8 = (qf[bh].T * 0.125).astype(np.float16)
        kt16 = kf[bh].T.astype(np.float16)
        qkt[j, :, 0:NSUM] = kf[bh][_SUMIDX].T.astype(np.float16)
        qkt[j, :, NSUM : NSUM + L] = qt8
        qkt[j, :, NSUM + L :] = kt16
        # odd windows of each pair, packed [64, 16*64]
        qko[j, :, 0 : L // 2] = (
            qt8.reshape(64, 16, 2, 64)[:, :, 1, :].reshape(64, L // 2)
        )
        qko[j, :, L // 2 :] = (
            kt16.reshape(64, 16, 2, 64)[:, :, 1, :].reshape(64, L // 2)
        )
        vp1 = np.concatenate([vf[bh], np.ones((L, 1), np.float32)], axis=1).astype(
            np.float16
        )
        vpx[j, :, :16, :] = vp1.reshape(16, 128, 65).transpose(1, 0, 2)
        vs1 = np.concatenate(
            [vf[bh][_SUMIDX], np.ones((NSUM, 1), np.float32)], axis=1
        ).astype(np.float16)
        vpx[j, :, 16:, :] = vs1.reshape(4, 128, 65).transpose(1, 0, 2)
    mall = np.concatenate([ms, mq, mk], axis=1)
    return {"qkt": qkt, "qko": qko, "vpx": vpx, "mall": mall, "m01": _host_m01()}


def _finish(o_raw):
    """[n, 65, L] unnormalized transposed device output -> [n, L, 64]."""
    o_raw = np.asarray(o_raw, np.float32)
    return (o_raw[:, :64, :] / o_raw[:, 64:65, :]).transpose(0, 2, 1)


def _in_maps(query, key, value):
    qf = np.asarray(query, np.float32).reshape(B * H, L, D)
    kf = np.asarray(key, np.float32).reshape(B * H, L, D)
    vf = np.asarray(value, np.float32).reshape(B * H, L, D)
    mq, mk, ms = _host_masks()
    return [
        _prep_core_inputs(qf, kf, vf, range(NBH * c, NBH * (c + 1)), mq, mk, ms)
        for c in range(NCORES)
    ]


def kernel(query, key, value):
    from concourse.bass_utils import run_bass_kernel_spmd

    nc = _build_nc()
    res = run_bass_kernel_spmd(nc, _in_maps(query, key, value), list(range(NCORES)))
    out = np.concatenate([_finish(res.results[c]["o"]) for c in range(NCORES)])
    return out.reshape(B, H, L, D).astype(np.float32)


# revision 31
# speedup vs baseline: 3.6978x; 1.0022x over previous
"""Block-sparse (DeepSpeed fixed-layout) causal self-attention on 8 trn2 NeuronCores.

Problem: B=2, H=16, L=2048, D=64, fp32; BLOCK=16, STRIDE=64, NUMVERTS=1, VERTSIZE=1.
Layout per head (identical for all heads since numverts=1):
  - intra-window block-causal attention within each 64-token window (4 blocks of 16)
  - "summary" attention: every query attends the last 16 tokens (block col 3) of
    every *earlier* 64-token window.

Strategy (per core; 32 (b,h) pairs sharded 4 per core, no collectives).
The Activation engine (exp, ~0.83 ns/column + ~143 ns/instr PSUM-access
penalty) is the binding resource, so the design minimizes exp'd columns and
exp instruction count; the PE side minimizes matmul instruction count
(small-N matmuls are issue-floor-bound):

  Summary QK:  St[sk,q] = lhsT.T @ rhs with
     lhsT = [summary K^T ; one-hot selector rows]   (stationary, fp16)
     rhs  = [Q^T/8 ; per-(window,block) mask-value rows]  (moving, fp16)
  -- the window-granularity causal masks are fused into the matmul as rank-8
  contraction rows per 128-key chunk.  The "active" mask value is -6*ln2 (not
  0) so exp() output is scaled 2^-6, keeping the unnormalized result in fp16
  range.  Chunks are computed into 2-bank PSUM pieces and exp'd with one
  fused-AP activation per piece (half the per-instr penalty of per-chunk exp).

  Packed local QK: the two 64-token windows of each window pair are computed
  by concurrent 64x64 PE tiles at tile_position (0,0)/(64,64) (even windows
  from partitions 0-63, odd windows from a second Q/K copy in partitions
  64-127), packing both windows' scores into one [128, 64] PSUM column range.
  This HALVES the local exp columns ([128,256] per group instead of [128,512]).
  The in-window causal mask (x 2^-6) is a DVE multiply producing two
  half-masked copies (etmA zeroes partitions 64-127, etmB zeroes 0-63) so the
  AV can contract the full 128 partitions with plain untiled matmuls --
  concurrent row-tiled matmuls would collide on the PSUM write port (HW fault).

  AV is transposed:  Ot[d,q] = [V|1].T @ Et  -- V (65 cols: 64 d + ones) is
  the *stationary* operand, the exp'd scores stream as the moving operand.
  This cuts AV to 18 matmuls per (b,h) and accumulates local + all summary
  chunks straight into one [65, 512] PSUM bank per query group.  Row 64 of Ot
  is the softmax denominator l.

  The device ships unnormalized Ot = [O^T ; l^T] fp16 (DVE copy PSUM->SBUF,
  per-group Pool-queue DMA); the host transposes and divides:
  O = (Ot[:64]/Ot[64]).T.  Startup DMAs land the first summary matmuls'
  operands ([summary K | group-0 Q] prefix) before the bulk; the last (b,h)
  runs its groups in reverse so the kernel ends on the shortest AV chain.
"""

import os
import numpy as np

# ---------------- problem constants (hardcoded per contract) ----------------
B, H, L, D = 2, 16, 2048, 64
BLOCK = 16
WIN = 64              # stride window (tokens)
NWIN = L // WIN       # 32 windows
NSUM = NWIN * BLOCK   # 512 summary keys (last 16 tokens of each window)
NG = 4                # query groups per sequence
GQ = L // NG          # 512 queries per group
NCORES = 8
NBH = (B * H) // NCORES  # 4 (b,h) per core
KP = 128              # contraction partitions: 64 d + 4 local mask + 32 tri mask + 28 zero
MASKVAL = -30000.0
EBIAS = -6.0 * float(np.log(2.0))  # exp() scale 2^-6: keeps unnorm output in fp16

_SUMIDX = np.array([64 * m + 48 + j for m in range(NWIN) for j in range(BLOCK)])


def _host_masks():
    """Constant mask rows appended to the contraction dim. fp16.

    Local attention is computed per *pair* of windows (128 keys x 128 queries).
    mq [64, L]    : mask *values* rows (appended to Q^T, the moving operand)
                    rows 0-7   = V8pair local-causal values (periodic 128)
                    rows 8-39  = V8_s triangular summary values (s = 0..3)
                    rows 40-63 = 0
    mk [64, L]    : mask *selector* rows appended to K^T (local stationary)
                    rows 0-7   = U8 one-hot of key 16-block within window pair
    ms [64, NSUM] : selector rows appended to the gathered summary K^T
                    rows 8+8s+b = one-hot of summary chunk s, block b
    Active (unmasked) entries carry EBIAS instead of 0 so every exp() result is
    scaled by 2^-6 (exactly one selector row fires per key column).
    """
    qc = np.arange(L)
    mq = np.zeros((64, L), np.float32)
    for s in range(4):
        for b in range(8):
            # summary block m=8s+b masked for queries in window <= m
            # (window granularity: same-window handled by the local path)
            mq[8 + 8 * s + b] = np.where(8 * s + b >= qc // WIN, MASKVAL, EBIAS)
    mk = np.zeros((64, L), np.float32)
    sc = np.arange(NSUM)
    ms = np.zeros((64, NSUM), np.float32)
    for s in range(4):
        for b in range(8):
            ms[8 + 8 * s + b] = ((sc // 128 == s) & ((sc % 128) // BLOCK == b)).astype(
                np.float32
            )
    return mq.astype(np.float16), mk.astype(np.float16), ms.astype(np.float16)


def _host_m01():
    """[128, 512] fp16 packed-local causal masks: partition p = key offset in
    window (two windows stacked), col j = (pair-in-group, query offset).
    Value 2^-6 where key block <= query block (the exp() 2^-6 scale rides the
    mask since packed-local scores carry no bias rows).  Cols 0:256 mask the
    A-window half (partitions 64-127 zeroed), cols 256:512 the B half -- the
    AV then contracts the full 128 partitions with plain untiled matmuls
    (concurrent row-tiled matmuls collide on the PSUM write port)."""
    p = np.arange(128)[:, None]
    j = np.arange(256)[None, :]
    valid = (p % 64) // BLOCK <= (j % 64) // BLOCK
    m = np.where(valid, 2.0 ** -6, 0.0)
    ma = np.where(p < 64, m, 0.0)
    mb = np.where(p >= 64, m, 0.0)
    return np.concatenate([ma, mb], axis=1).astype(np.float16)


# ---------------- device program ----------------
_NC_CACHE = {}


def _build_nc(reps=1):
    """reps>1 repeats the whole computation in-NEFF (timing only)."""
    if reps in _NC_CACHE:
        return _NC_CACHE[reps]
    from contextlib import ExitStack

    import concourse.bacc as bacc
    import concourse.bass as bass
    import concourse.tile as tile
    from concourse import mybir

    F16 = mybir.dt.float16
    F32 = mybir.dt.float32
    EXP = mybir.ActivationFunctionType.Exp

    nc = bacc.Bacc("TRN2", target_bir_lowering=False)

    # qkt = [gathered-summary K^T | Q^T/8 | K^T] concatenated along cols
    # (summary K first so the startup-critical DMA is one contiguous prefix)
    qkt_d = nc.dram_tensor("qkt", [NBH, 64, 2 * L + NSUM], F16, kind="ExternalInput")
    # vpx = [V|1] reshaped (16 local 128-key tiles) ++ gathered summary [V|1]
    # (4 tiles) -> one tensor, one DMA per (b,h)
    vpx_d = nc.dram_tensor("vpx", [NBH, 128, 20, 65], F16, kind="ExternalInput")
    mall_d = nc.dram_tensor("mall", [64, 2 * L + NSUM], F16, kind="ExternalInput")
    # odd-window Q^T/8 ++ K^T, landed in partitions 64-127 of the K region
    # (the packed-local 64x64 PE tiles need both windows of a pair in
    # disjoint partition halves)
    qko_d = nc.dram_tensor("qko", [NBH, 64, 2 * L // 2], F16, kind="ExternalInput")
    m01_d = nc.dram_tensor("m01", [128, 512], F16, kind="ExternalInput")
    # unnormalized transposed output; row 64 = softmax denominator l (host
    # transposes + divides)
    o_d = nc.dram_tensor("o", [NBH, 65, L], F16, kind="ExternalOutput")

    with tile.TileContext(nc) as tc, ExitStack() as ctx:
        const = ctx.enter_context(tc.tile_pool(name="const", bufs=1))
        inbuf = ctx.enter_context(tc.tile_pool(name="inbuf", bufs=2))
        etp = ctx.enter_context(tc.tile_pool(name="etp", bufs=3))
        etsum = ctx.enter_context(tc.tile_pool(name="etsum", bufs=2))
        psum = ctx.enter_context(tc.tile_pool(name="psum", bufs=2, space="PSUM"))
        outp = ctx.enter_context(tc.tile_pool(name="outp", bufs=2))

        # double-buffered wide base [Q^T | K^T | KTS] with persistent mask rows
        qktb = [const.tile([KP, 2 * L + NSUM], F16, name=f"qktb{j}") for j in range(2)]
        m01t = const.tile([128, 512], F16, name="m01t")

        def bh_setup(rep, i):
            """Emit input DMAs for (rep, i); return the per-bh view dict.
            The mask rows (mall) only cover [0 : NSUM+L) -- the old mk region
            is dead and overwritten by qko, and overlapping DMAs would chain
            a WAW wait into the SP DMA FIFO."""
            qkt = qktb[i % 2]
            if rep == 0 and i == 0:
                # critical-path startup: land the prefix the first summary
                # matmuls need ([KTS | Q group 0] + its mask rows) first
                src = qkt_d.ap()[i]
                msrc = mall_d.ap()
                cut = NSUM + GQ
                nc.scalar.dma_start(out=qkt[64:128, 0:cut], in_=msrc[:, 0:cut])
                nc.gpsimd.dma_start(out=qkt[0:64, 0:cut], in_=src[:, 0:cut])
                nc.gpsimd.dma_start(out=m01t, in_=m01_d.ap())
                nc.sync.dma_start(
                    out=qkt[64:128, cut : NSUM + L], in_=msrc[:, cut : NSUM + L]
                )
                # group-0 local QK needs the odd-window data early
                nc.sync.dma_start(
                    out=qkt[64:128, NSUM + L : NSUM + 2 * L], in_=qko_d.ap()[i]
                )
                nc.sync.dma_start(out=qkt[0:64, cut:], in_=src[:, cut:])
            else:
                if rep == 0 and i == 1:
                    # first use of the second buffer: load its mask rows
                    nc.sync.dma_start(
                        out=qkt[64:128, 0 : NSUM + L], in_=mall_d.ap()[:, 0 : NSUM + L]
                    )
                nc.sync.dma_start(out=qkt[0:64, :], in_=qkt_d.ap()[i])
            if not (rep == 0 and i == 0):
                nc.sync.dma_start(
                    out=qkt[64:128, NSUM + L : NSUM + 2 * L], in_=qko_d.ap()[i]
                )
            vpx = inbuf.tile([128, 20, 65], F16, tag="vpx")
            nc.sync.dma_start(out=vpx, in_=vpx_d.ap()[i])
            osb = outp.tile([128, NG, GQ], F16, tag="osb", name=f"osb_{rep}_{i}")
            return dict(
                kts=qkt[:, 0:NSUM],
                qt=qkt[:, NSUM : NSUM + L],
                kt=qkt[:, NSUM + L : NSUM + 2 * L],
                qo=qkt[:, NSUM + L : NSUM + L + L // 2],      # partitions 64-127
                ko=qkt[:, NSUM + L + L // 2 : NSUM + 2 * L],  # partitions 64-127
                vpx=vpx,
                osb=osb,
            )

        # software-pipelined (b,h) x group unit stream: the next bh's first
        # (lightest) group is emitted before the current bh's last (heaviest)
        # group so ACT never starves at bh boundaries; the final bh runs its
        # groups in reverse so the kernel ends on the shortest AV chain.
        def unit_stream(rep):
            stream = []
            for i in range(NBH):
                gs = [0, 1, 2, 3] if i < NBH - 1 else [3, 2, 1, 0]
                stream.extend((i, g) for g in gs)
            return stream

        for rep in range(reps):
            views = {}
            for i, g in unit_stream(rep):
                if i not in views:
                    views[i] = bh_setup(rep, i)
                v = views[i]
                kts, qt, kt, qo, ko = v["kts"], v["qt"], v["kt"], v["qo"], v["ko"]
                vpx, osb = v["vpx"], v["osb"]
                # ---- summary QK (chunks s = 0..g) + exp fused per 2-chunk
                # piece: halves ACT's per-instr PSUM access penalty while
                # keeping the pipeline fine-grained ----
                ets = []
                for a in range(0, g + 1, 2):
                    b = min(a + 2, g + 1)
                    st = psum.tile(
                        [128, 2, GQ], F32, tag="st_sum",
                        name=f"st_{rep}_{i}_{g}_{a}", bufs=2,
                    )
                    for s in range(a, b):
                        nc.tensor.matmul(
                            st[:, s - a, :],
                            kts[:, 128 * s : 128 * (s + 1)],
                            qt[:, GQ * g : GQ * (g + 1)],
                            start=True,
                            stop=True,
                            skip_group_check=True,
                        )
                    e = etsum.tile(
                        [128, 2, GQ], F16, tag="et_sum",
                        name=f"et_{rep}_{i}_{g}_{a}", bufs=6,
                    )
                    nc.scalar.activation(
                        out=e[:, 0 : b - a, :], in_=st[:, 0 : b - a, :], func=EXP
                    )
                    ets.extend(e[:, s - a, :] for s in range(a, b))

                # ---- packed local QK: two 64-token windows per pair in
                # disjoint partition halves via 64x64 PE tiles (halves the
                # local exp columns) ----
                stl = psum.tile([128, 256], F32, tag="st_loc", name=f"stl_{rep}_{i}_{g}")
                for u in range(4):
                    p = 4 * g + u
                    nc.tensor.matmul(
                        stl[0:64, 64 * u : 64 * (u + 1)],
                        kt[0:64, 128 * p : 128 * p + 64],
                        qt[0:64, 128 * p : 128 * p + 64],
                        start=True,
                        stop=True,
                        skip_group_check=True,
                    )
                    nc.tensor.matmul(
                        stl[64:128, 64 * u : 64 * (u + 1)],
                        ko[64:128, 64 * p : 64 * (p + 1)],
                        qo[64:128, 64 * p : 64 * (p + 1)],
                        start=True,
                        stop=True,
                        skip_group_check=True,
                    )
                etl = etp.tile([128, 256], F16, tag="et_loc", name=f"etl_{rep}_{i}_{g}")
                nc.scalar.activation(out=etl, in_=stl, func=EXP)
                # causal-in-window masks (x 2^-6 exp scale), DVE 4x mode;
                # A/B halves also zero the other window's partitions
                etma = etp.tile([128, 256], F16, tag="et_mska", name=f"etma_{rep}_{i}_{g}")
                nc.vector.tensor_mul(etma, etl, m01t[:, 0:256])
                etmb = etp.tile([128, 256], F16, tag="et_mskb", name=f"etmb_{rep}_{i}_{g}")
                nc.vector.tensor_mul(etmb, etl, m01t[:, 256:512])

                # ---- transposed AV: Ot[65, 512] = [V|1].T @ Et, V stationary ----
                # start=True only on the first matmul: it clears the whole
                # bank's has_written bits; u>0 then overwrite their (cleared)
                # ranges with start=False, and the summary matmuls accumulate.
                ot = psum.tile(
                    [128, GQ], F32, tag="otps", name=f"ot_{rep}_{i}_{g}", bufs=2
                )
                for u in range(4):
                    nc.tensor.matmul(
                        ot[0:65, 128 * u : 128 * u + 64],
                        vpx[:, 4 * g + u, :],
                        etma[:, 64 * u : 64 * (u + 1)],
                        start=(u == 0),
                        stop=False,
                        skip_group_check=True,
                    )
                    nc.tensor.matmul(
                        ot[0:65, 128 * u + 64 : 128 * (u + 1)],
                        vpx[:, 4 * g + u, :],
                        etmb[:, 64 * u : 64 * (u + 1)],
                        start=False,
                        stop=False,
                        skip_group_check=True,
                    )
                for s in range(g + 1):
                    nc.tensor.matmul(
                        ot[0:65, :],
                        vpx[:, 16 + s, :],
                        ets[s],
                        start=False,
                        stop=(s == g),
                        skip_group_check=True,
                    )

                # ---- move unnormalized Ot to SBUF fp16 (host divides) ----
                nc.vector.tensor_copy(out=osb[0:65, g, :], in_=ot[0:65, :])
                nc.gpsimd.dma_start(
                    out=o_d.ap()[i][:, GQ * g : GQ * (g + 1)], in_=osb[0:65, g, :]
                )

    nc.compile()
    _NC_CACHE[reps] = nc
    return nc


def _prep_core_inputs(qf, kf, vf, bhs, mq, mk, ms):
    """Build one core's input dict from flat [32, L, D] fp32 arrays."""
    qkt = np.empty((NBH, 64, 2 * L + NSUM), np.float16)
    qko = np.empty((NBH, 64, L), np.float16)
    vpx = np.empty((NBH, 128, 20, 65), np.float16)
    for j, bh in enumerate(bhs):
        qt8 = (qf[bh].T * 0.125).astype(np.float16)
        kt16 = kf[bh].T.astype(np.float16)
        qkt[j, :, 0:NSUM] = kf[bh][_SUMIDX].T.astype(np.float16)
        qkt[j, :, NSUM : NSUM + L] = qt8
        qkt[j, :, NSUM + L :] = kt16
        # odd windows of each pair, packed [64, 16*64]
        qko[j, :, 0 : L // 2] = (
            qt8.reshape(64, 16, 2, 64)[:, :, 1, :].reshape(64, L // 2)
        )
        qko[j, :, L // 2 :] = (
            kt16.reshape(64, 16, 2, 64)[:, :, 1, :].reshape(64, L // 2)
        )
        vp1 = np.concatenate([vf[bh], np.ones((L, 1), np.float32)], axis=1).astype(
            np.float16
        )
        vpx[j, :, :16, :] = vp1.reshape(16, 128, 65).transpose(1, 0, 2)
        vs1 = np.concatenate(
            [vf[bh][_SUMIDX], np.ones((NSUM, 1), np.float32)], axis=1
        ).astype(np.float16)
        vpx[j, :, 16:, :] = vs1.reshape(4, 128, 65).transpose(1, 0, 2)
    mall = np.concatenate([ms, mq, mk], axis=1)
    return {"qkt": qkt, "qko": qko, "vpx": vpx, "mall": mall, "m01": _host_m01()}


def _finish(o_raw):
    """[n, 65, L] unnormalized transposed device output -> [n, L, 64]."""
    o_raw = np.asarray(o_raw, np.float32)
    return (o_raw[:, :64, :] / o_raw[:, 64:65, :]).transpose(0, 2, 1)


def _in_maps(query, key, value):
    qf = np.asarray(query, np.float32).reshape(B * H, L, D)
    kf = np.asarray(key, np.float32).reshape(B * H, L, D)
    vf = np.asarray(value, np.float32).reshape(B * H, L, D)
    mq, mk, ms = _host_masks()
    return [
        _prep_core_inputs(qf, kf, vf, range(NBH * c, NBH * (c + 1)), mq, mk, ms)
        for c in range(NCORES)
    ]


def kernel(query, key, value):
    from concourse.bass_utils import run_bass_kernel_spmd

    nc = _build_nc()
    res = run_bass_kernel_spmd(nc, _in_maps(query, key, value), list(range(NCORES)))
    out = np.concatenate([_finish(res.results[c]["o"]) for c in range(NCORES)])
    return out.reshape(B, H, L, D).astype(np.float32)
